# revision 40
# baseline (speedup 1.0000x reference)
"""Trainium2 Bass kernel for nn_Attention_3599182594919.

Multi-head attention, B=8 N=2048 C=384 H=6 D=64, data-parallel over batch
across 8 NeuronCores (one batch element per core, no collectives).

Per-core algorithm (layouts chosen so no on-chip transposes are needed and
every DVE op keeps all operands on the same partition window):
  host:  xT = x[b].T bf16                              [C, N]
         emt[k, q] = exp(-1e5*(mask[b][q,k] - min_k mask[b][q,:]))  bf16 [N, N]
           (softmax max-shift folded into the mask factor; exp(a+b)=exp(a)*exp(b))
  dev:   qkT = [Wq; Wk] @ xT                           [2C, N] bf16 (partition = feature)
         v_aug = [x[b] @ Wv.T | 1]                     [N, H, D+1] bf16
         per head pair hp, q-tile qt (512 q):
           S^T[k, q] = kT.T @ qT           (PE bf16, even head rows 0-63 /
                                            odd head rows 64-127, concurrent)
           e = exp(0.125 * S^T)            (ACT, multi-bank PSUM read, bf16 out)
           P = e * emt                     (DVE bf16 2x)
           [O^T; l] += [v|1].T @ P         (PE bf16, PSUM accum over k-tiles)
           ao_h = O^T[0:64] * (1/l)        (l broadcast to 64 partitions via a
                                            K=1 matmul from partition 64, then
                                            reciprocal + multiply on DVE)
         y[tok, :] = sum_h ao_h.T @ pw_h + b            (PE, 6 K=64 matmuls)

PSUM: pool A [128,2048] (4 banks, 2 kt-pairs) and pool B [128,1024] (2 banks,
1 kt-pair) alternate so the ACT exp of one group overlaps the S matmuls of the
next; pvp (2 banks) holds the two PV accumulators.
"""

from contextlib import ExitStack

import numpy as np
import ml_dtypes

import concourse.bass as bass
import concourse.mybir as mybir
from concourse import bacc
from concourse.tile import TileContext
from concourse.bass_utils import run_bass_kernel_spmd

F32 = mybir.dt.float32
BF16 = mybir.dt.bfloat16

B, N, C, H = 8, 2048, 384, 6
D = C // H          # 64
QT = N // 512       # 4  q-tiles of 512
KT = N // 128       # 16 k-tiles of 128
NT = N // 128       # 16 token tiles

# set by test harness to capture timing
TRACE = False
LAST_RESULT = None

_NC_CACHE = None


def build_nc():
    nc = bacc.Bacc("TRN2", target_bir_lowering=False, debug=False)

    xT = nc.declare_dram_parameter("xT", [C, N], BF16, isOutput=False)
    wqkT = nc.declare_dram_parameter("wqkT", [C, 2 * C], BF16, isOutput=False)
    wvT = nc.declare_dram_parameter("wvT", [C, C], BF16, isOutput=False)
    pwT = nc.declare_dram_parameter("pwT", [C, C], BF16, isOutput=False)
    pb = nc.declare_dram_parameter("pb", [1, C], BF16, isOutput=False)
    emt = nc.declare_dram_parameter("emt", [N, N], BF16, isOutput=False)
    out = nc.declare_dram_parameter("out", [N, C], F32, isOutput=True)

    emt_r = emt.ap().rearrange("(t p) q -> p t q", p=128)  # [128, KT, N]

    with TileContext(nc) as tc:
        with ExitStack() as ctx:
            consts = ctx.enter_context(tc.tile_pool(name="consts", bufs=1))
            wpool = ctx.enter_context(tc.tile_pool(name="weights", bufs=1))
            qkpool = ctx.enter_context(tc.tile_pool(name="qk", bufs=1))
            vpool = ctx.enter_context(tc.tile_pool(name="v", bufs=1))
            psA = ctx.enter_context(tc.tile_pool(name="psA", bufs=2, space="PSUM"))
            pvp = ctx.enter_context(tc.tile_pool(name="pvp", bufs=2, space="PSUM"))
            yep = ctx.enter_context(tc.tile_pool(name="yep", bufs=2, space="PSUM"))

            # ---- constants ----
            ones_row = consts.tile([128, 64], BF16, tag="ones_row")
            nc.vector.memset(ones_row[:, :], 1.0)
            ones1 = consts.tile([1, 128], BF16, tag="ones1")
            nc.vector.memset(ones1[:, :], 1.0)
            pb_sb = consts.tile([1, C], BF16, tag="pbsb")
            nc.sync.dma_start(out=pb_sb[:, :], in_=pb[:, :])

            # ---- load x^T and weights ----
            xtpool = ctx.enter_context(tc.tile_pool(name="xt", bufs=1))

            def load(t, src):
                nc.sync.dma_start(out=t[:, :], in_=src)

            xT_sb = []
            for i in range(3):
                t = xtpool.tile([128, N], BF16, tag=f"xT{i}")
                load(t, xT[i * 128:(i + 1) * 128, :])
                xT_sb.append(t)
            wqkT_sb = []
            for i in range(3):
                t = wpool.tile([128, 2 * C], BF16, tag=f"wqk{i}")
                load(t, wqkT[i * 128:(i + 1) * 128, :])
                wqkT_sb.append(t)
            wvT_sb = []
            for i in range(3):
                t = wpool.tile([128, C], BF16, tag=f"wv{i}")
                load(t, wvT[i * 128:(i + 1) * 128, :])
                wvT_sb.append(t)
            # proj weights: not needed until the first finalize chunks, so
            # issued as one merged DMA after the critical loads (see below)
            pw_all = wpool.tile([64, H, C], BF16, tag="pwall")
            pw6_sb = [pw_all[:, h, :] for h in range(H)]

            # ---- PE warm-up: dummy matmuls with no data deps run during the
            # initial DMAs so the HAM clock gate is at 8/8 when phase B starts.
            warm_ps = yep.tile([64, 64], F32, tag="ye", name="warm_ps")
            for _ in range(100):
                nc.tensor.matmul(
                    warm_ps[:, :], ones_row[:, :], ones_row[:, :],
                    start=True, stop=True,
                )

            emtp = ctx.enter_context(tc.tile_pool(name="emtp", bufs=8))
            epool = ctx.enter_context(tc.tile_pool(name="e", bufs=4))
            ppool = ctx.enter_context(tc.tile_pool(name="p", bufs=4))
            aopool = ctx.enter_context(tc.tile_pool(name="ao", bufs=14))
            lpool = ctx.enter_context(tc.tile_pool(name="l", bufs=4))
            ypool = ctx.enter_context(tc.tile_pool(name="y", bufs=3))

            # ---- phases B/C as single-chain emitters (interleaved into qt0's
            # attention stream): qkT = [Wq; Wk] @ xT and v_aug = [x Wv.T | 1]
            qkT_sb = []
            for f in range(6):
                qkT_sb.append(qkpool.tile([128, N], BF16, name=f"qk{f}", tag=f"qk{f}"))
            vaug_sb = []
            for t_i in range(NT):
                vaug_sb.append(
                    vpool.tile([128, H, D + 1], BF16, name=f"va{t_i}", tag=f"va{t_i}")
                )
            bc_state = [0]

            vec_only = [True]

            def evac(dst, src):
                if vec_only[0] or bc_state[0] % 2 == 0:
                    nc.vector.tensor_copy(dst, src)
                else:
                    nc.scalar.copy(dst, src)

            def qk_chain(f, j, pools=((pvp, "pv"), (yep, "ye"))):
                pool, tg = pools[bc_state[0] % len(pools)]
                ps = pool.tile([128, 512], F32, tag=tg, name=f"qkps{f}_{j}")
                for kc in range(3):
                    nc.tensor.matmul(
                        ps[:, :],
                        wqkT_sb[kc][:, f * 128:(f + 1) * 128],
                        xT_sb[kc][:, j * 512:(j + 1) * 512],
                        start=(kc == 0),
                        stop=(kc == 2),
                    )
                evac(qkT_sb[f][:, j * 512:(j + 1) * 512], ps[:, :])
                bc_state[0] += 1

            def v_chain(t_i, pools=((pvp, "pv"), (yep, "ye"))):
                pool, tg = pools[bc_state[0] % len(pools)]
                ps = pool.tile([128, C], F32, tag=tg, name=f"vps{t_i}")
                for kc in range(3):
                    nc.tensor.matmul(
                        ps[:, :],
                        xT_sb[kc][:, t_i * 128:(t_i + 1) * 128],
                        wvT_sb[kc][:, :],
                        start=(kc == 0),
                        stop=(kc == 2),
                    )
                nc.gpsimd.memset(vaug_sb[t_i][:, :, :], 1.0)
                evac(vaug_sb[t_i][:, :, 0:D],
                     ps[:, :].rearrange("p (h d) -> p h d", d=D))
                bc_state[0] += 1

            # upfront: just what (qt0, hp0) needs to start — q/k features for
            # heads 0/1 and the first few v tiles
            # absolute minimum before the first S/exp groups: q features for
            # heads 0/1 over q-tile 0 (f0 j0), k features for kt 0-3 (f3 j0),
            # and the first v tiles
            for f, j in ((0, 0), (3, 0)):
                qk_chain(f, j)
            for t_i in (0, 1, 2):
                v_chain(t_i)

            # remaining projections drain inside qt0's attention (yep pool only,
            # since pvp holds the PV accumulators there); later k-feature and
            # q-feature chains stay just ahead of their first use
            vec_only[0] = False
            yo = ((yep, "ye"),)
            init_hp0 = []
            qk_rest0 = [(3, 1), (3, 2), (3, 3), (0, 1), (0, 2), (0, 3)] + \
                       [(1, j) for j in range(QT)] + [(4, j) for j in range(QT)]
            for idx, t_i in enumerate(range(3, NT)):
                if idx < len(qk_rest0):
                    f, j = qk_rest0[idx]
                    init_hp0.append(lambda f=f, j=j: qk_chain(f, j, yo))
                init_hp0.append(lambda t=t_i: v_chain(t, yo))
            for f, j in qk_rest0[NT - 3:]:
                init_hp0.append(lambda f=f, j=j: qk_chain(f, j, yo))
            init_hp1 = [lambda f=f, j=j: qk_chain(f, j, yo)
                        for f in (2, 5) for j in range(QT)]

            def s_mm(sp_slice, hp, head_off, kt, qt):
                """One S^T matmul: [64,128] kT (stationary) x [64,512] qT."""
                nc.tensor.matmul(
                    sp_slice,
                    qkT_sb[3 + hp][head_off:head_off + 64, kt * 128:(kt + 1) * 128],
                    qkT_sb[hp][head_off:head_off + 64, qt * 512:(qt + 1) * 512],
                    start=True,
                    stop=True,
                )

            def pv_mm(pv_ps, hp, par, kt, pt_slice):
                nc.tensor.matmul(
                    pv_ps[0:D + 1, :],
                    vaug_sb[kt][:, 2 * hp + par, :],
                    pt_slice,
                    start=(kt == 0),
                    stop=(kt == KT - 1),
                )

            def norm_chunk(ao_tiles, pvu, h, fqt):
                # broadcast l from partition 64 down to partitions 0-63
                # with a K=1 matmul, reciprocal straight from PSUM, mult.
                bc = yep.tile([128, 512], F32, tag="ye", name=f"bc{fqt}_{h}")
                nc.tensor.matmul(
                    bc[0:64, :], ones_row[64:65, :], pvu[64:65, :],
                    start=True, stop=True,
                )
                bcs = lpool.tile([64, 512], F32, tag="bcs", name=f"bcs{fqt}_{h}")
                nc.vector.reciprocal_approx_fast(out=bcs[:, :], in_=bc[0:64, :])
                ao = aopool.tile([64, 512], BF16, tag="ao", name=f"ao{fqt}_{h}")
                nc.vector.tensor_mul(ao[:, :], pvu[0:64, :], bcs[:, :])
                ao_tiles[h] = ao

            def proj_chunk(ao_tiles, tt, fqt):
                # y = sum_h ao_h.T @ pw_h + b for token tile tt
                ps = yep.tile([128, C], F32, tag="ye", name=f"yps{fqt}_{tt}")
                for h in range(H):
                    nc.tensor.matmul(
                        ps[:, :],
                        ao_tiles[h][:, tt * 128:(tt + 1) * 128],
                        pw6_sb[h][:, :],
                        start=(h == 0),
                        stop=False,
                    )
                nc.tensor.matmul(
                    ps[:, :], ones1[:, :], pb_sb[:, :], start=False, stop=True
                )
                yt = ypool.tile([128, C], F32, tag="y", name=f"yt{fqt}_{tt}")
                nc.vector.tensor_copy(yt[:, :], ps[:, :])
                row = (fqt * 4 + tt) * 128
                nc.sync.dma_start(out=out[row:row + 128, :], in_=yt[:, :])

            # ---- phase D: attention per q-tile, with the previous q-tile's
            # normalization + output projection interleaved into this one's
            # emission so no engine head-of-line blocks on the serial chain ----
            em_map = {}

            def emit_em(eqt):
                tiles = []
                for kg in range(4):
                    em = emtp.tile([128, 4 * 512], BF16, tag="emt",
                                   name=f"em{eqt}_{kg}")
                    nc.sync.dma_start(
                        out=em[:, :],
                        in_=emt_r[:, kg * 4:(kg + 1) * 4, eqt * 512:(eqt + 1) * 512],
                    )
                    tiles.append(em)
                em_map[eqt] = tiles

            pending_chunks = []
            pv_lag = []
            flush_chunks = []
            emit_em(0)
            nc.sync.dma_start(
                out=pw_all[:, :, :],
                in_=pwT.ap().rearrange("(h p) c -> p h c", p=64),
            )
            for qt in range(QT):
                em_tiles = em_map.pop(qt)

                ao_cur = [None] * H
                for hp in range(3):
                    pv_e = pvp.tile([128, 512], F32, tag="pv")
                    pv_o = pvp.tile([128, 512], F32, tag="pv")
                    for kt in range(KT):
                        if qt == 0 and hp == 0:
                            for _ in range(2):
                                if init_hp0:
                                    init_hp0.pop(0)()
                        if qt == 0 and hp == 1 and init_hp1:
                            init_hp1.pop(0)()
                        if hp == 1 and kt == 0 and qt + 1 < QT:
                            emit_em(qt + 1)
                        sp = psA.tile([128, 1024], F32, tag="sA")
                        s_mm(sp[:, 0:512], hp, 0, kt, qt)
                        s_mm(sp[:, 512:1024], hp, 64, kt, qt)
                        et = epool.tile([128, 1024], BF16, tag="eA")
                        nc.scalar.activation(
                            et[:, :], sp[:, :],
                            mybir.ActivationFunctionType.Exp, scale=0.125,
                        )
                        pt = ppool.tile([128, 1024], BF16, tag="pA")
                        emsl = em_tiles[kt // 4][:, (kt % 4) * 512:((kt % 4) + 1) * 512]
                        emsl2 = bass.AP(
                            tensor=emsl.tensor,
                            offset=emsl.offset,
                            ap=[emsl.ap[0], [0, 2]] + list(emsl.ap[1:]),
                        )
                        nc.vector.tensor_mul(
                            pt[:, :].rearrange("p (two q) -> p two q", two=2),
                            et[:, :].rearrange("p (two q) -> p two q", two=2),
                            emsl2,
                        )
                        # PV matmuls lag 2 groups behind so the next head
                        # pair's S matmuls never sit behind a PV that waits
                        # on this group's exp/multiply chain.
                        pv_lag.append((pv_e, pv_o, hp, kt, pt))
                        if len(pv_lag) > 2:
                            e_, o_, hp_, kt_, pt_ = pv_lag.pop(0)
                            pv_mm(e_, hp_, 0, kt_, pt_[:, 0:512])
                            pv_mm(o_, hp_, 1, kt_, pt_[:, 512:1024])
                            if kt_ == KT - 1 and flush_chunks:
                                flush_chunks.pop(0)()
                        if pending_chunks and kt % 4 == 3:
                            pending_chunks.pop(0)()

                    # Evacuate [O^T; l] to SBUF immediately (bf16) so the
                    # PSUM accumulators free up for the next head pair.
                    pvu_e = aopool.tile([65, 512], BF16, tag="pvu", name=f"pvu{qt}_{hp}e")
                    pvu_o = aopool.tile([65, 512], BF16, tag="pvu", name=f"pvu{qt}_{hp}o")
                    flush_chunks.append(
                        lambda e=pv_e, o=pv_o, ue=pvu_e, uo=pvu_o: (
                            nc.vector.tensor_copy(ue[:, :], e[0:D + 1, :]),
                            nc.vector.tensor_copy(uo[:, :], o[0:D + 1, :]),
                        )
                    )
                    pending_chunks.append(
                        lambda a=ao_cur, p=pvu_e, h=2 * hp, q=qt: norm_chunk(a, p, h, q)
                    )
                    pending_chunks.append(
                        lambda a=ao_cur, p=pvu_o, h=2 * hp + 1, q=qt: norm_chunk(a, p, h, q)
                    )

                for tt in range(4):
                    pending_chunks.append(
                        lambda a=ao_cur, t=tt, q=qt: proj_chunk(a, t, q)
                    )

            # drain the final head pair's PV matmuls + evacuations
            while pv_lag:
                e_, o_, hp_, kt_, pt_ = pv_lag.pop(0)
                pv_mm(e_, hp_, 0, kt_, pt_[:, 0:512])
                pv_mm(o_, hp_, 1, kt_, pt_[:, 512:1024])
            for fc in flush_chunks:
                fc()
            # drain the final q-tile's normalization + projection
            for ch in pending_chunks:
                ch()

    nc.compile()
    return nc


def _get_nc():
    global _NC_CACHE
    if _NC_CACHE is None:
        _NC_CACHE = build_nc()
    return _NC_CACHE


def kernel(**inputs):
    x = np.asarray(inputs["x"], dtype=np.float32)
    mask = np.asarray(inputs["mask"], dtype=np.float32)
    qkv_w = np.asarray(inputs["qkv_w"], dtype=np.float32)
    proj_w = np.asarray(inputs["proj_w"], dtype=np.float32)
    proj_b = np.asarray(inputs["proj_b"], dtype=np.float32)

    nc = _get_nc()

    bf16 = ml_dtypes.bfloat16
    wqkT = np.ascontiguousarray(qkv_w[:2 * C].T.astype(bf16))
    wvT = np.ascontiguousarray(qkv_w[2 * C:].T.astype(bf16))
    pwT = np.ascontiguousarray(proj_w.T.astype(bf16))
    pb = np.ascontiguousarray(proj_b.reshape(1, C).astype(bf16))

    in_maps = []
    for b in range(B):
        xTb = np.ascontiguousarray(x[b].T.astype(bf16))
        mm = mask[b] - mask[b].min(axis=1, keepdims=True)
        emtb = np.exp(-1e5 * mm).T.astype(bf16)
        in_maps.append(
            {
                "xT": xTb,
                "wqkT": wqkT,
                "wvT": wvT,
                "pwT": pwT,
                "pb": pb,
                "emt": np.ascontiguousarray(emtb),
            }
        )

    global LAST_RESULT
    res = run_bass_kernel_spmd(nc, in_maps, core_ids=list(range(B)), trace=TRACE)
    LAST_RESULT = res
    return np.stack([res.results[b]["out"] for b in range(B)]).astype(np.float32)


# revision 47
# speedup vs baseline: 1.0700x; 1.0700x over previous
"""Trainium2 Bass kernel for nn_Attention_3599182594919.

Multi-head attention, B=8 N=2048 C=384 H=6 D=64, data-parallel over batch
across 8 NeuronCores (one batch element per core, no collectives).

Per-core algorithm (layouts chosen so no on-chip transposes are needed and
every DVE op keeps all operands on the same partition window):
  host:  xT = x[b].T bf16                              [C, N]
         emt[k, q] = exp(-1e5*(mask[b][q,k] - min_k mask[b][q,:]))  bf16 [N, N]
           (softmax max-shift folded into the mask factor; exp(a+b)=exp(a)*exp(b))
  dev:   qkT = [Wq; Wk] @ xT                           [2C, N] bf16 (partition = feature)
         v_aug = [x[b] @ Wv.T | 1]                     [N, H, D+1] bf16
         per head pair hp, q-tile qt (512 q):
           S^T[k, q] = kT.T @ qT           (PE bf16, even head rows 0-63 /
                                            odd head rows 64-127, concurrent)
           e = exp(0.125 * S^T)            (ACT, multi-bank PSUM read, bf16 out)
           P = e * emt                     (DVE bf16 2x)
           [O^T; l] += [v|1].T @ P         (PE bf16, PSUM accum over k-tiles)
           ao_h = O^T[0:64] * (1/l)        (l broadcast to 64 partitions via a
                                            K=1 matmul from partition 64, then
                                            reciprocal + multiply on DVE)
         y[tok, :] = sum_h ao_h.T @ pw_h + b            (PE, 6 K=64 matmuls)

PSUM: pool A [128,2048] (4 banks, 2 kt-pairs) and pool B [128,1024] (2 banks,
1 kt-pair) alternate so the ACT exp of one group overlaps the S matmuls of the
next; pvp (2 banks) holds the two PV accumulators.
"""

from contextlib import ExitStack

import numpy as np
import ml_dtypes

import concourse.bass as bass
import concourse.mybir as mybir
from concourse import bacc
from concourse.tile import TileContext
from concourse.bass_utils import run_bass_kernel_spmd

F32 = mybir.dt.float32
BF16 = mybir.dt.bfloat16

B, N, C, H = 8, 2048, 384, 6
D = C // H          # 64
QT = N // 512       # 4  q-tiles of 512
KT = N // 128       # 16 k-tiles of 128
NT = N // 128       # 16 token tiles

# set by test harness to capture timing
TRACE = False
LAST_RESULT = None

_NC_CACHE = None


def build_nc():
    nc = bacc.Bacc("TRN2", target_bir_lowering=False, debug=False)

    xT = nc.declare_dram_parameter("xT", [C, N], BF16, isOutput=False)
    wqkT = nc.declare_dram_parameter("wqkT", [C, 2 * C], BF16, isOutput=False)
    wvT = nc.declare_dram_parameter("wvT", [C, C], BF16, isOutput=False)
    pwT = nc.declare_dram_parameter("pwT", [C, C], BF16, isOutput=False)
    pb = nc.declare_dram_parameter("pb", [1, C], BF16, isOutput=False)
    emt = nc.declare_dram_parameter("emt", [N, N], BF16, isOutput=False)
    out = nc.declare_dram_parameter("out", [N, C], F32, isOutput=True)

    emt_r = emt.ap().rearrange("(t p) q -> p t q", p=128)  # [128, KT, N]

    with TileContext(nc) as tc:
        with ExitStack() as ctx:
            consts = ctx.enter_context(tc.tile_pool(name="consts", bufs=1))
            wpool = ctx.enter_context(tc.tile_pool(name="weights", bufs=1))
            qkpool = ctx.enter_context(tc.tile_pool(name="qk", bufs=1))
            vpool = ctx.enter_context(tc.tile_pool(name="v", bufs=1))
            psA = ctx.enter_context(tc.tile_pool(name="psA", bufs=2, space="PSUM"))
            pvp = ctx.enter_context(tc.tile_pool(name="pvp", bufs=2, space="PSUM"))
            yep = ctx.enter_context(tc.tile_pool(name="yep", bufs=2, space="PSUM"))

            # ---- constants ----
            ones_row = consts.tile([128, 64], BF16, tag="ones_row")
            nc.vector.memset(ones_row[:, :], 1.0)
            ones1 = consts.tile([1, 128], BF16, tag="ones1")
            nc.vector.memset(ones1[:, :], 1.0)
            pb_sb = consts.tile([1, C], BF16, tag="pbsb")
            nc.sync.dma_start(out=pb_sb[:, :], in_=pb[:, :])

            # ---- load x^T and weights ----
            xtpool = ctx.enter_context(tc.tile_pool(name="xt", bufs=1))

            def load(t, src):
                nc.sync.dma_start(out=t[:, :], in_=src)

            xT_sb = []
            for i in range(3):
                t = xtpool.tile([128, N], BF16, tag=f"xT{i}")
                load(t, xT[i * 128:(i + 1) * 128, :])
                xT_sb.append(t)
            wqkT_sb = []
            for i in range(3):
                t = wpool.tile([128, 2 * C], BF16, tag=f"wqk{i}")
                load(t, wqkT[i * 128:(i + 1) * 128, :])
                wqkT_sb.append(t)
            wvT_sb = []
            for i in range(3):
                t = wpool.tile([128, C], BF16, tag=f"wv{i}")
                load(t, wvT[i * 128:(i + 1) * 128, :])
                wvT_sb.append(t)
            # proj weights: not needed until the first finalize chunks, so
            # issued as one merged DMA after the critical loads (see below)
            pw_all = wpool.tile([64, H, C], BF16, tag="pwall")
            pw6_sb = [pw_all[:, h, :] for h in range(H)]

            # ---- PE warm-up: dummy matmuls with no data deps run during the
            # initial DMAs so the HAM clock gate is at 8/8 when phase B starts.
            warm_ps = yep.tile([64, 64], F32, tag="ye", name="warm_ps")
            for _ in range(100):
                nc.tensor.matmul(
                    warm_ps[:, :], ones_row[:, :], ones_row[:, :],
                    start=True, stop=True,
                )

            emtp = ctx.enter_context(tc.tile_pool(name="emtp", bufs=8))
            epool = ctx.enter_context(tc.tile_pool(name="e", bufs=4))
            ppool = ctx.enter_context(tc.tile_pool(name="p", bufs=15))
            aopool = ctx.enter_context(tc.tile_pool(name="ao", bufs=14))
            lpool = ctx.enter_context(tc.tile_pool(name="l", bufs=4))
            ypool = ctx.enter_context(tc.tile_pool(name="y", bufs=3))

            # ---- phases B/C as single-chain emitters (interleaved into qt0's
            # attention stream): qkT = [Wq; Wk] @ xT and v_aug = [x Wv.T | 1]
            qkT_sb = []
            for f in range(6):
                qkT_sb.append(qkpool.tile([128, N], BF16, name=f"qk{f}", tag=f"qk{f}"))
            vaug_sb = []
            for t_i in range(NT):
                vaug_sb.append(
                    vpool.tile([128, H, D + 1], BF16, name=f"va{t_i}", tag=f"va{t_i}")
                )
            bc_state = [0]

            vec_only = [True]

            def evac(dst, src):
                if vec_only[0] or bc_state[0] % 2 == 0:
                    nc.vector.tensor_copy(dst, src)
                else:
                    nc.scalar.copy(dst, src)

            def qk_chain(f, j, pools=((pvp, "pv"), (yep, "ye"))):
                pool, tg = pools[bc_state[0] % len(pools)]
                ps = pool.tile([128, 512], F32, tag=tg, name=f"qkps{f}_{j}")
                for kc in range(3):
                    nc.tensor.matmul(
                        ps[:, :],
                        wqkT_sb[kc][:, f * 128:(f + 1) * 128],
                        xT_sb[kc][:, j * 512:(j + 1) * 512],
                        start=(kc == 0),
                        stop=(kc == 2),
                    )
                evac(qkT_sb[f][:, j * 512:(j + 1) * 512], ps[:, :])
                bc_state[0] += 1

            def v_chain(t_i, pools=((pvp, "pv"), (yep, "ye"))):
                pool, tg = pools[bc_state[0] % len(pools)]
                ps = pool.tile([128, C], F32, tag=tg, name=f"vps{t_i}")
                for kc in range(3):
                    nc.tensor.matmul(
                        ps[:, :],
                        xT_sb[kc][:, t_i * 128:(t_i + 1) * 128],
                        wvT_sb[kc][:, :],
                        start=(kc == 0),
                        stop=(kc == 2),
                    )
                nc.gpsimd.memset(vaug_sb[t_i][:, :, :], 1.0)
                evac(vaug_sb[t_i][:, :, 0:D],
                     ps[:, :].rearrange("p (h d) -> p h d", d=D))
                bc_state[0] += 1

            # upfront: just what (qt0, hp0) needs to start — q/k features for
            # heads 0/1 and the first few v tiles
            # absolute minimum before the first S/exp groups: q features for
            # heads 0/1 over q-tile 0 (f0 j0), k features for kt 0-3 (f3 j0),
            # and the first v tiles
            for f, j in ((0, 0), (3, 0)):
                qk_chain(f, j)
            for t_i in (0, 1, 2):
                v_chain(t_i)

            # remaining projections drain inside qt0's attention (yep pool only,
            # since pvp holds the PV accumulators there); later k-feature and
            # q-feature chains stay just ahead of their first use
            vec_only[0] = False
            yo = ((yep, "ye"),)
            init_hp0 = []
            qk_rest0 = [(3, 1), (3, 2), (3, 3), (0, 1), (0, 2), (0, 3)] + \
                       [(1, j) for j in range(QT)] + [(4, j) for j in range(QT)]
            for idx, t_i in enumerate(range(3, NT)):
                if idx < len(qk_rest0):
                    f, j = qk_rest0[idx]
                    init_hp0.append(lambda f=f, j=j: qk_chain(f, j, yo))
                init_hp0.append(lambda t=t_i: v_chain(t, yo))
            for f, j in qk_rest0[NT - 3:]:
                init_hp0.append(lambda f=f, j=j: qk_chain(f, j, yo))
            init_hp1 = [lambda f=f, j=j: qk_chain(f, j, yo)
                        for f in (2, 5) for j in range(QT)]

            def s_mm(sp_slice, hp, head_off, kt, qt):
                """One S^T matmul: [64,128] kT (stationary) x [64,512] qT."""
                nc.tensor.matmul(
                    sp_slice,
                    qkT_sb[3 + hp][head_off:head_off + 64, kt * 128:(kt + 1) * 128],
                    qkT_sb[hp][head_off:head_off + 64, qt * 512:(qt + 1) * 512],
                    start=True,
                    stop=True,
                )

            def pv_mm(pv_ps, hp, par, kt, pt_slice):
                nc.tensor.matmul(
                    pv_ps[0:D + 1, :],
                    vaug_sb[kt][:, 2 * hp + par, :],
                    pt_slice,
                    start=(kt == 0),
                    stop=(kt == KT - 1),
                )

            def norm_chunk(ao_tiles, pvu, h, fqt):
                # broadcast l from partition 64 down to partitions 0-63
                # with a K=1 matmul, reciprocal straight from PSUM, mult.
                bc = yep.tile([128, 512], F32, tag="ye", name=f"bc{fqt}_{h}")
                nc.tensor.matmul(
                    bc[0:64, :], ones_row[64:65, :], pvu[64:65, :],
                    start=True, stop=True,
                )
                bcs = lpool.tile([64, 512], F32, tag="bcs", name=f"bcs{fqt}_{h}")
                nc.vector.reciprocal_approx_fast(out=bcs[:, :], in_=bc[0:64, :])
                ao = aopool.tile([64, 512], BF16, tag="ao", name=f"ao{fqt}_{h}")
                nc.vector.tensor_mul(ao[:, :], pvu[0:64, :], bcs[:, :])
                ao_tiles[h] = ao

            def proj_chunk(ao_tiles, tt, fqt):
                # y = sum_h ao_h.T @ pw_h + b for token tile tt
                ps = yep.tile([128, C], F32, tag="ye", name=f"yps{fqt}_{tt}")
                for h in range(H):
                    nc.tensor.matmul(
                        ps[:, :],
                        ao_tiles[h][:, tt * 128:(tt + 1) * 128],
                        pw6_sb[h][:, :],
                        start=(h == 0),
                        stop=False,
                    )
                nc.tensor.matmul(
                    ps[:, :], ones1[:, :], pb_sb[:, :], start=False, stop=True
                )
                yt = ypool.tile([128, C], F32, tag="y", name=f"yt{fqt}_{tt}")
                nc.vector.tensor_copy(yt[:, :], ps[:, :])
                row = (fqt * 4 + tt) * 128
                nc.sync.dma_start(out=out[row:row + 128, :], in_=yt[:, :])

            # ---- phase D: attention per q-tile, with the previous q-tile's
            # normalization + output projection interleaved into this one's
            # emission so no engine head-of-line blocks on the serial chain ----
            em_map = {}

            def emit_em(eqt):
                tiles = []
                for kg in range(4):
                    em = emtp.tile([128, 4 * 512], BF16, tag="emt",
                                   name=f"em{eqt}_{kg}")
                    nc.sync.dma_start(
                        out=em[:, :],
                        in_=emt_r[:, kg * 4:(kg + 1) * 4, eqt * 512:(eqt + 1) * 512],
                    )
                    tiles.append(em)
                em_map[eqt] = tiles

            pending_chunks = []   # entries: (required_flush_count, emit_fn)
            pv_lag = []
            flush_chunks = []
            flush_done = [0]
            pair_count = [0]
            emit_em(0)
            nc.sync.dma_start(
                out=pw_all[:, :, :],
                in_=pwT.ap().rearrange("(h p) c -> p h c", p=64),
            )
            for qt in range(QT):
                em_tiles = em_map.pop(qt)

                ao_cur = [None] * H
                for hp in range(3):
                    pv_e = pvp.tile([128, 512], F32, tag="pv")
                    pv_o = pvp.tile([128, 512], F32, tag="pv")
                    for kt in range(KT):
                        if qt == 0 and hp == 0:
                            for _ in range(2):
                                if init_hp0:
                                    init_hp0.pop(0)()
                        if qt == 0 and hp == 1 and init_hp1:
                            init_hp1.pop(0)()
                        if hp == 1 and kt == 0 and qt + 1 < QT:
                            emit_em(qt + 1)
                        sp = psA.tile([128, 1024], F32, tag="sA")
                        s_mm(sp[:, 0:512], hp, 0, kt, qt)
                        s_mm(sp[:, 512:1024], hp, 64, kt, qt)
                        et = epool.tile([128, 1024], BF16, tag="eA")
                        nc.scalar.activation(
                            et[:, :], sp[:, :],
                            mybir.ActivationFunctionType.Exp, scale=0.125,
                        )
                        pt = ppool.tile([128, 1024], BF16, tag="pA")
                        emsl = em_tiles[kt // 4][:, (kt % 4) * 512:((kt % 4) + 1) * 512]
                        emsl2 = bass.AP(
                            tensor=emsl.tensor,
                            offset=emsl.offset,
                            ap=[emsl.ap[0], [0, 2]] + list(emsl.ap[1:]),
                        )
                        nc.vector.tensor_mul(
                            pt[:, :].rearrange("p (two q) -> p two q", two=2),
                            et[:, :].rearrange("p (two q) -> p two q", two=2),
                            emsl2,
                        )
                        # PV matmuls lag 2 groups behind so the next head
                        # pair's S matmuls never sit behind a PV that waits
                        # on this group's exp/multiply chain.
                        pv_lag.append((pv_e, pv_o, hp, kt, pt))
                        if len(pv_lag) > 12:
                            e_, o_, hp_, kt_, pt_ = pv_lag.pop(0)
                            pv_mm(e_, hp_, 0, kt_, pt_[:, 0:512])
                            pv_mm(o_, hp_, 1, kt_, pt_[:, 512:1024])
                            if kt_ == KT - 1 and flush_chunks:
                                flush_chunks.pop(0)()
                                flush_done[0] += 1
                        if (pending_chunks and kt % 4 == 3
                                and pending_chunks[0][0] <= flush_done[0]):
                            pending_chunks.pop(0)[1]()

                    # Evacuate [O^T; l] to SBUF immediately (bf16) so the
                    # PSUM accumulators free up for the next head pair.
                    pvu_e = aopool.tile([65, 512], BF16, tag="pvu", name=f"pvu{qt}_{hp}e")
                    pvu_o = aopool.tile([65, 512], BF16, tag="pvu", name=f"pvu{qt}_{hp}o")
                    flush_chunks.append(
                        lambda e=pv_e, o=pv_o, ue=pvu_e, uo=pvu_o: (
                            nc.vector.tensor_copy(ue[:, :], e[0:D + 1, :]),
                            nc.vector.tensor_copy(uo[:, :], o[0:D + 1, :]),
                        )
                    )
                    pair_count[0] += 1
                    req = pair_count[0]
                    pending_chunks.append(
                        (req, lambda a=ao_cur, p=pvu_e, h=2 * hp, q=qt: norm_chunk(a, p, h, q))
                    )
                    pending_chunks.append(
                        (req, lambda a=ao_cur, p=pvu_o, h=2 * hp + 1, q=qt: norm_chunk(a, p, h, q))
                    )

                for tt in range(4):
                    pending_chunks.append(
                        (3 * (qt + 1), lambda a=ao_cur, t=tt, q=qt: proj_chunk(a, t, q))
                    )

            # drain the final head pair's PV matmuls + evacuations
            while pv_lag:
                e_, o_, hp_, kt_, pt_ = pv_lag.pop(0)
                pv_mm(e_, hp_, 0, kt_, pt_[:, 0:512])
                pv_mm(o_, hp_, 1, kt_, pt_[:, 512:1024])
            for fc in flush_chunks:
                fc()
                flush_done[0] += 1
            # drain the final q-tile's normalization + projection
            for _, ch in pending_chunks:
                ch()

    nc.compile()
    return nc


def _get_nc():
    global _NC_CACHE
    if _NC_CACHE is None:
        _NC_CACHE = build_nc()
    return _NC_CACHE


def kernel(**inputs):
    x = np.asarray(inputs["x"], dtype=np.float32)
    mask = np.asarray(inputs["mask"], dtype=np.float32)
    qkv_w = np.asarray(inputs["qkv_w"], dtype=np.float32)
    proj_w = np.asarray(inputs["proj_w"], dtype=np.float32)
    proj_b = np.asarray(inputs["proj_b"], dtype=np.float32)

    nc = _get_nc()

    bf16 = ml_dtypes.bfloat16
    wqkT = np.ascontiguousarray(qkv_w[:2 * C].T.astype(bf16))
    wvT = np.ascontiguousarray(qkv_w[2 * C:].T.astype(bf16))
    pwT = np.ascontiguousarray(proj_w.T.astype(bf16))
    pb = np.ascontiguousarray(proj_b.reshape(1, C).astype(bf16))

    in_maps = []
    for b in range(B):
        xTb = np.ascontiguousarray(x[b].T.astype(bf16))
        mm = mask[b] - mask[b].min(axis=1, keepdims=True)
        emtb = np.exp(-1e5 * mm).T.astype(bf16)
        in_maps.append(
            {
                "xT": xTb,
                "wqkT": wqkT,
                "wvT": wvT,
                "pwT": pwT,
                "pb": pb,
                "emt": np.ascontiguousarray(emtb),
            }
        )

    global LAST_RESULT
    res = run_bass_kernel_spmd(nc, in_maps, core_ids=list(range(B)), trace=TRACE)
    LAST_RESULT = res
    return np.stack([res.results[b]["out"] for b in range(B)]).astype(np.float32)
